# revision 1
# baseline (speedup 1.0000x reference)
"""GQA attention kernel for 8 Trainium2 NeuronCores.

Sharding: core = (batch b, kv_group g), b in {0,1}, g in {0..3}.
Each core computes the 4 heads of one KV group for one batch and the
partial output projection for those heads; the host sums the 4 group
partials per batch.  Zero duplicated compute across cores.

Per-core layout choices (all matmuls run in float32r = full PE rate):
  - host passes xT = x[b].T so every projection has contraction on
    partitions without any on-device transpose of x
  - QT/KT are produced directly in [head_dim, S] layout; V in natural
    [S, head_dim] layout (via a PE transpose of VT)
  - scoresT[t, q] = KT_tile^T @ QT  -> exp on ACT (no max subtraction:
    scores are ~N(0,1) after folding 1/sqrt(D) into Wq, exp is safe)
  - softmax denominators via an all-ones stationary matmul (partition
    reduction on PE); the redundant 128 identical rows make the
    reciprocal + normalize plain full-tile DVE ops (no broadcasts)
  - attention output is accumulated transposed (outT[d, q]) so the
    output projection needs no transpose either; the host transposes
    the final [E, S] partial back to [S, E].
"""

import numpy as np

# problem shape (hardcoded per contract)
B, S, E = 2, 2048, 2048
H, G, D = 16, 4, 128
R = H // G          # heads per kv group = 4
KV = G * D          # 512
ST = S // 128       # 16 t-tiles
ET = E // 128       # 16 e-tiles
SC = S // 512       # 4 s-chunks
NPAIR = S // 1024   # 2 q-chunk pairs

_cache = {}


def _split_multi_waits(nc, maxw=1):
    """Walrus in this container accepts only one sync-wait per
    instruction; move extra waits onto preceding same-engine NoOps."""
    from concourse import mybir

    n_split = 0
    for fn in nc.m.functions:
        for bb in fn.blocks:
            out = []
            changed = False
            for inst in bb.instructions:
                si = inst.sync_info
                waits = list(si.on_wait or []) if si is not None else []
                if len(waits) > maxw:
                    changed = True
                    n_split += 1
                    head, tail = waits[:-maxw], waits[-maxw:]
                    for j in range(0, len(head), maxw):
                        nop = mybir.InstNoOp(
                            name=f"{inst.name}-wsplit{j}", ins=[], outs=[]
                        )
                        nop.engine = inst.engine
                        nop.sync_info = mybir.SyncInfo(
                            on_wait=head[j : j + maxw], on_update=[]
                        )
                        out.append(nop)
                    si.on_wait = tail
                out.append(inst)
            if changed:
                bb.instructions = out
    return n_split


def _build_program():
    import concourse.bass as bass
    import concourse.tile as tile
    from concourse import mybir
    from concourse.masks import make_identity

    F32R = mybir.dt.float32r
    F32 = mybir.dt.float32
    Exp = mybir.ActivationFunctionType.Exp
    Mult = mybir.AluOpType.mult

    nc = bass.Bass(target_bir_lowering=False)

    xT = nc.dram_tensor("xT", [E, S], F32R, kind="ExternalInput")
    wq = nc.dram_tensor("wq", [E, R * D], F32R, kind="ExternalInput")
    wk = nc.dram_tensor("wk", [E, D], F32R, kind="ExternalInput")
    wv = nc.dram_tensor("wv", [E, D], F32R, kind="ExternalInput")
    wo = nc.dram_tensor("wo", [R * D, E], F32R, kind="ExternalInput")
    bqv = nc.dram_tensor("bqv", [R * D], F32, kind="ExternalInput")
    bkv = nc.dram_tensor("bkv", [D], F32, kind="ExternalInput")
    bvv = nc.dram_tensor("bvv", [D], F32, kind="ExternalInput")
    otd = nc.dram_tensor("ot", [E, S], F32, kind="ExternalOutput")

    with tile.TileContext(nc) as tc:
        import contextlib

        with contextlib.ExitStack() as ctx:
            consts = ctx.enter_context(tc.tile_pool(name="consts", bufs=1))
            qkvt = ctx.enter_context(tc.tile_pool(name="qkvt", bufs=1))

            ident_f = consts.tile([128, 128], F32)
            make_identity(nc, ident_f)
            ident = consts.tile([128, 128], F32R)
            nc.vector.tensor_copy(ident, ident_f)
            ones_f = consts.tile([128, 128], F32)
            nc.gpsimd.memset(ones_f, 1.0)
            ones = consts.tile([128, 128], F32R)
            nc.vector.tensor_copy(ones, ones_f)
            bq_sb = consts.tile([128, R], F32)
            nc.sync.dma_start(bq_sb, bqv.rearrange("(o p) -> p o", p=128))
            bk_sb = consts.tile([128, 1], F32)
            nc.sync.dma_start(bk_sb, bkv.rearrange("(o p) -> p o", p=128))
            bv_sb = consts.tile([128, 1], F32)
            nc.sync.dma_start(bv_sb, bvv.rearrange("(o p) -> p o", p=128))

            QT = qkvt.tile([128, R, S], F32R)    # QT[d, h, s]
            KT = qkvt.tile([128, S], F32R)       # KT[d, t]
            V = qkvt.tile([128, ST, D], F32R)    # V[t%128, tt, d]

            # ---- phase 1: QKV^T projections + V transpose ----
            with tc.tile_pool(name="wts", bufs=1) as wpool, \
                 tc.tile_pool(name="xts", bufs=2) as xtpool, \
                 tc.tile_pool(name="vt", bufs=1) as vtpool, \
                 tc.tile_pool(name="ps1", bufs=3, space="PSUM") as ps1, \
                 tc.tile_pool(name="psv", bufs=2, space="PSUM") as psv:
                wq_sb = wpool.tile([128, ET, R * D], F32R)
                nc.sync.dma_start(wq_sb, wq.rearrange("(o p) m -> p o m", p=128))
                wk_sb = wpool.tile([128, ET, D], F32R)
                nc.sync.dma_start(wk_sb, wk.rearrange("(o p) m -> p o m", p=128))
                wv_sb = wpool.tile([128, ET, D], F32R)
                nc.sync.dma_start(wv_sb, wv.rearrange("(o p) m -> p o m", p=128))
                VT = vtpool.tile([128, S], F32R)

                for sc in range(SC):
                    xtile = xtpool.tile([128, ET, 512], F32R, tag="xt")
                    for e in range(ET):
                        nc.sync.dma_start(
                            xtile[:, e],
                            xT[e * 128 : (e + 1) * 128, sc * 512 : (sc + 1) * 512],
                        )
                    cs = slice(sc * 512, (sc + 1) * 512)
                    for ot in range(R + 2):
                        psum = ps1.tile([128, 512], F32, tag="p1")
                        for e in range(ET):
                            if ot < R:
                                lhsT = wq_sb[:, e, ot * 128 : (ot + 1) * 128]
                            elif ot == R:
                                lhsT = wk_sb[:, e]
                            else:
                                lhsT = wv_sb[:, e]
                            nc.tensor.matmul(
                                psum, lhsT, xtile[:, e],
                                start=(e == 0), stop=(e == ET - 1),
                            )
                        if ot < R:
                            nc.scalar.add(QT[:, ot, cs], psum, bq_sb[:, ot : ot + 1])
                        elif ot == R:
                            nc.scalar.add(KT[:, cs], psum, bk_sb[:, 0:1])
                        else:
                            nc.scalar.add(VT[:, cs], psum, bv_sb[:, 0:1])

                for tt in range(ST):
                    ps = psv.tile([128, 128], F32R, tag="pv")
                    nc.tensor.transpose(ps, VT[:, tt * 128 : (tt + 1) * 128], ident)
                    nc.vector.tensor_copy(V[:, tt], ps)

            # ---- phase 2: attention per head ----
            p23 = ctx.enter_context(tc.tile_pool(name="p23", bufs=1))
            outT = p23.tile([128, R, S], F32R)  # normalized attn outT[d, h, s]
            wo_sb = p23.tile([128, R, E], F32R)
            nc.sync.dma_start(wo_sb, wo.rearrange("(o p) m -> p o m", p=128))
            with tc.tile_pool(name="probs", bufs=3) as probs_pool, \
                 tc.tile_pool(name="recip", bufs=2) as rpool, \
                 tc.tile_pool(name="ps_s", bufs=2, space="PSUM") as ps_s, \
                 tc.tile_pool(name="ps_sum", bufs=1, space="PSUM") as ps_sum, \
                 tc.tile_pool(name="ps_av", bufs=1, space="PSUM") as ps_av:

                for h in range(R):
                    for pr in range(NPAIR):
                        q0 = pr * 1024
                        sums_ps = ps_sum.tile([128, 1024], F32, tag="sums")
                        out_ps = ps_av.tile([128, 1024], F32, tag="av")
                        for tt in range(ST):
                            pss = ps_s.tile([128, 1024], F32, tag="scores")
                            kslice = KT[:, tt * 128 : (tt + 1) * 128]
                            for hf in range(2):
                                nc.tensor.matmul(
                                    pss[:, hf * 512 : (hf + 1) * 512],
                                    kslice,
                                    QT[:, h, q0 + hf * 512 : q0 + (hf + 1) * 512],
                                    start=True, stop=True,
                                )
                            pt = probs_pool.tile([128, 1024], F32R, tag="probs")
                            nc.scalar.activation(pt, pss, Exp)
                            for hf in range(2):
                                hs = slice(hf * 512, (hf + 1) * 512)
                                nc.tensor.matmul(
                                    sums_ps[:, hs], ones, pt[:, hs],
                                    start=(tt == 0), stop=(tt == ST - 1),
                                )
                                nc.tensor.matmul(
                                    out_ps[:, hs], V[:, tt], pt[:, hs],
                                    start=(tt == 0), stop=(tt == ST - 1),
                                )
                        rc = rpool.tile([128, 1024], F32, tag="recip")
                        nc.vector.reciprocal(rc, sums_ps)
                        nc.vector.tensor_tensor(
                            outT[:, h, q0 : q0 + 1024], out_ps, rc, Mult
                        )

            # ---- phase 3: output projection (transposed) ----
            with tc.tile_pool(name="ostage", bufs=3) as ostage, \
                 tc.tile_pool(name="ps_o", bufs=4, space="PSUM") as ps_o:
                for et in range(ET):
                    for sc in range(SC):
                        ps = ps_o.tile([128, 512], F32, tag="po")
                        for h in range(R):
                            nc.tensor.matmul(
                                ps,
                                wo_sb[:, h, et * 128 : (et + 1) * 128],
                                outT[:, h, sc * 512 : (sc + 1) * 512],
                                start=(h == 0), stop=(h == R - 1),
                            )
                        st = ostage.tile([128, 512], F32, tag="ost")
                        nc.vector.tensor_copy(st, ps)
                        nc.sync.dma_start(
                            otd[et * 128 : (et + 1) * 128,
                                sc * 512 : (sc + 1) * 512],
                            st,
                        )

    _split_multi_waits(nc)
    return nc


def _prepare(x, Wq, bq, Wk, bk, Wv, bv, Wo, bo):
    """Host-side sharding: build per-core input maps."""
    x = np.asarray(x, dtype=np.float32)
    Wq = np.asarray(Wq, dtype=np.float32)
    bq = np.asarray(bq, dtype=np.float32)
    Wk = np.asarray(Wk, dtype=np.float32)
    bk = np.asarray(bk, dtype=np.float32)
    Wv = np.asarray(Wv, dtype=np.float32)
    bv = np.asarray(bv, dtype=np.float32)
    Wo = np.asarray(Wo, dtype=np.float32)

    isd = np.float32(1.0 / np.sqrt(D))
    xTs = [np.ascontiguousarray(x[b].T) for b in range(B)]
    in_maps = []
    for core in range(8):
        b, g = divmod(core, G)
        in_maps.append({
            "xT": xTs[b],
            "wq": np.ascontiguousarray(Wq[:, g * R * D : (g + 1) * R * D]) * isd,
            "wk": np.ascontiguousarray(Wk[:, g * D : (g + 1) * D]),
            "wv": np.ascontiguousarray(Wv[:, g * D : (g + 1) * D]),
            "wo": np.ascontiguousarray(Wo[g * R * D : (g + 1) * R * D, :]),
            "bqv": bq[g * R * D : (g + 1) * R * D] * isd,
            "bkv": bk[g * D : (g + 1) * D],
            "bvv": bv[g * D : (g + 1) * D],
        })
    return in_maps


def _gather(results, bo):
    bo = np.asarray(bo, dtype=np.float32)
    out = np.empty((B, S, E), dtype=np.float32)
    for b in range(B):
        acc = results[b * G]["ot"].copy()
        for g in range(1, G):
            acc += results[b * G + g]["ot"]
        out[b] = acc.T + bo
    return out


def kernel(x, Wq, bq, Wk, bk, Wv, bv, Wo, bo):
    from concourse.bass_utils import run_bass_kernel_spmd

    if "nc" not in _cache:
        _cache["nc"] = _build_program()
    nc = _cache["nc"]
    in_maps = _prepare(x, Wq, bq, Wk, bk, Wv, bv, Wo, bo)
    res = run_bass_kernel_spmd(nc, in_maps, core_ids=list(range(8)))
    return _gather(res.results, bo)



# revision 4
# speedup vs baseline: 1.3457x; 1.3457x over previous
"""GQA attention kernel for 8 Trainium2 NeuronCores.

Sharding: core = (batch b, kv_group g), b in {0,1}, g in {0..3}.
Each core computes the 4 heads of one KV group for one batch and the
partial output projection for those heads; the host sums the 4 group
partials per batch.  Zero duplicated compute across cores.

All matmul operands are bf16 (fp32 PSUM accumulation); verified to give
~5e-3 max rel err vs the fp32 reference (tolerance 2e-2).

Performance structure (the tensor engine is the bottleneck, so the
whole kernel is organized to keep its queue dense and stall-free):
  - phase 1 (QKV projections): e-tile innermost with 6 concurrent PSUM
    accumulation groups, so compute starts after the first 128KB of
    DMA instead of the full 2MB chunk; per-e-tile weight DMAs
    interleaved with x-tile DMAs.  V is transposed by the DMA xbar
    (dma_start_transpose), costing no PE/DVE time.
  - phase 2 (attention): two head-streams interleaved at kv-tile
    granularity, with the scores matmul issued one kv-tile AHEAD of
    the sums/AV matmuls that consume exp(scores).  The ACT engine's
    exp latency is thereby hidden behind 5 other matmuls, so the PE
    never waits (the baseline stalled ~0.9us per kv-tile).
  - softmax denominators: ones-stationary matmul (partition reduction
    on PE); PSUM banks are drained by fast ACT copies so the 2+2+4
    PSUM bank rotation never blocks; the slow reciprocal runs
    off-critical-path on DVE (reciprocal_approx_fast, 18-bit).
  - phase 3 (output projection): transposed accumulation as in
    baseline; output DMA overlaps the projection matmuls.
"""

import numpy as np

# problem shape (hardcoded per contract)
B, S, E = 2, 2048, 2048
H, G, D = 16, 4, 128
R = H // G          # heads per kv group = 4
ST = S // 128       # 16 t-tiles
ET = E // 128       # 16 e-tiles
SC = S // 512       # 4 s-chunks
QC = S // 512       # 4 q-chunks
NO = R + 2          # projection outputs per e-tile: 4x Q slices, K, V

_cache = {}


def _split_multi_waits(nc, maxw=1):
    """Walrus in this container accepts only one sync-wait per
    instruction; move extra waits onto preceding same-engine NoOps."""
    from concourse import mybir

    n_split = 0
    for fn in nc.m.functions:
        for bb in fn.blocks:
            out = []
            changed = False
            for inst in bb.instructions:
                si = inst.sync_info
                waits = list(si.on_wait or []) if si is not None else []
                if len(waits) > maxw:
                    changed = True
                    n_split += 1
                    head, tail = waits[:-maxw], waits[-maxw:]
                    for j in range(0, len(head), maxw):
                        nop = mybir.InstNoOp(
                            name=f"{inst.name}-wsplit{j}", ins=[], outs=[]
                        )
                        nop.engine = inst.engine
                        nop.sync_info = mybir.SyncInfo(
                            on_wait=head[j : j + maxw], on_update=[]
                        )
                        out.append(nop)
                    si.on_wait = tail
                out.append(inst)
            if changed:
                bb.instructions = out
    return n_split


def _build_program():
    import contextlib

    import concourse.bass as bass
    import concourse.tile as tile
    from concourse import mybir

    BF16 = mybir.dt.bfloat16
    F32 = mybir.dt.float32
    Exp = mybir.ActivationFunctionType.Exp
    Copy = mybir.ActivationFunctionType.Copy
    Mult = mybir.AluOpType.mult

    nc = bass.Bass(target_bir_lowering=False)

    xT = nc.dram_tensor("xT", [E, S], BF16, kind="ExternalInput")
    wq = nc.dram_tensor("wq", [E, R * D], BF16, kind="ExternalInput")
    wk = nc.dram_tensor("wk", [E, D], BF16, kind="ExternalInput")
    wv = nc.dram_tensor("wv", [E, D], BF16, kind="ExternalInput")
    wo = nc.dram_tensor("wo", [R * D, E], BF16, kind="ExternalInput")
    bqv = nc.dram_tensor("bqv", [R * D], F32, kind="ExternalInput")
    bkv = nc.dram_tensor("bkv", [D], F32, kind="ExternalInput")
    bvv = nc.dram_tensor("bvv", [D], F32, kind="ExternalInput")
    otd = nc.dram_tensor("ot", [E, S], F32, kind="ExternalOutput")

    with tile.TileContext(nc) as tc:
        with contextlib.ExitStack() as ctx:
            consts = ctx.enter_context(tc.tile_pool(name="consts", bufs=1))
            big = ctx.enter_context(tc.tile_pool(name="big", bufs=1))

            bq_sb = consts.tile([128, R], F32)
            nc.sync.dma_start(bq_sb, bqv.rearrange("(o p) -> p o", p=128))
            bk_sb = consts.tile([128, 1], F32)
            nc.sync.dma_start(bk_sb, bkv.rearrange("(o p) -> p o", p=128))
            bv_sb = consts.tile([128, 1], F32)
            nc.sync.dma_start(bv_sb, bvv.rearrange("(o p) -> p o", p=128))

            ones_f = consts.tile([128, 128], F32)
            nc.gpsimd.memset(ones_f, 1.0)
            ones = consts.tile([128, 128], BF16)
            nc.vector.tensor_copy(ones, ones_f)

            QT = big.tile([128, R, S], BF16)    # QT[d, h, q]
            KT = big.tile([128, S], BF16)       # KT[d, t]
            VT = big.tile([128, S], BF16)       # VT[d, t]
            V = big.tile([128, ST, D], BF16)    # V[t%128, tt, d]
            outT = big.tile([128, R, S], BF16)  # normalized attn out
            wo_sb = big.tile([128, R, E], BF16)

            # ---- phase 1: QKV^T projections ----
            with tc.tile_pool(name="wts", bufs=1) as wpool, \
                 tc.tile_pool(name="xts", bufs=2) as xtpool, \
                 tc.tile_pool(name="ps1", bufs=8, space="PSUM") as ps1:
                wq_sb = wpool.tile([128, ET, R * D], BF16)
                wk_sb = wpool.tile([128, ET, D], BF16)
                wv_sb = wpool.tile([128, ET, D], BF16)
                # interleave per-e weight DMAs with the first x chunk so the
                # first matmul is gated on ~0.3MB of DMA, not 4.5MB
                xt0 = xtpool.tile([128, ET, 512], BF16, tag="xt")
                for e in range(ET):
                    re = slice(e * 128, (e + 1) * 128)
                    nc.sync.dma_start(wq_sb[:, e], wq[re, :])
                    nc.sync.dma_start(wk_sb[:, e], wk[re, :])
                    nc.sync.dma_start(wv_sb[:, e], wv[re, :])
                    nc.sync.dma_start(xt0[:, e], xT[re, 0:512])

                for sc in range(SC):
                    if sc == 0:
                        xtile = xt0
                    else:
                        xtile = xtpool.tile([128, ET, 512], BF16, tag="xt")
                        for e in range(ET):
                            nc.sync.dma_start(
                                xtile[:, e],
                                xT[e * 128 : (e + 1) * 128,
                                   sc * 512 : (sc + 1) * 512],
                            )
                    cs = slice(sc * 512, (sc + 1) * 512)
                    pss = [ps1.tile([128, 512], F32, tag="p1",
                                    name=f"p1_{sc}_{i}")
                           for i in range(NO)]
                    for e in range(ET):
                        for ot in range(NO):
                            if ot < R:
                                lhsT = wq_sb[:, e, ot * 128 : (ot + 1) * 128]
                            elif ot == R:
                                lhsT = wk_sb[:, e]
                            else:
                                lhsT = wv_sb[:, e]
                            nc.tensor.matmul(
                                pss[ot], lhsT, xtile[:, e],
                                start=(e == 0), stop=(e == ET - 1),
                            )
                    for ot in range(NO):
                        if ot < R:
                            nc.scalar.add(QT[:, ot, cs], pss[ot],
                                          bq_sb[:, ot : ot + 1])
                        elif ot == R:
                            nc.scalar.add(KT[:, cs], pss[ot], bk_sb[:, 0:1])
                        else:
                            nc.scalar.add(VT[:, cs], pss[ot], bv_sb[:, 0:1])
                    # V transpose for this chunk's 4 t-tiles via the DMA xbar
                    for tt in range(sc * 4, sc * 4 + 4):
                        nc.sync.dma_start_transpose(
                            V[:, tt], VT[:, tt * 128 : (tt + 1) * 128]
                        )

                # wo is needed only in phase 3; enqueue after everything else
                nc.sync.dma_start(wo_sb, wo.rearrange("(o p) m -> p o m", p=128))

            # ---- phase 2: attention, two head-streams interleaved ----
            with tc.tile_pool(name="probs", bufs=6) as ppool, \
                 tc.tile_pool(name="ssb", bufs=4) as spool, \
                 tc.tile_pool(name="avsb", bufs=4) as avspool, \
                 tc.tile_pool(name="rcs", bufs=4) as rpool, \
                 tc.tile_pool(name="ps_sc", bufs=4, space="PSUM") as scpool, \
                 tc.tile_pool(name="ps_sum", bufs=2, space="PSUM") as smpool, \
                 tc.tile_pool(name="ps_av", bufs=2, space="PSUM") as avpool:
                for qc in range(QC):
                    qs = slice(qc * 512, (qc + 1) * 512)
                    for hp in range(R // 2):
                        hA, hB = 2 * hp, 2 * hp + 1
                        sums_A = smpool.tile([128, 512], F32, tag="sums")
                        sums_B = smpool.tile([128, 512], F32, tag="sums")
                        av_A = avpool.tile([128, 512], F32, tag="av")
                        av_B = avpool.tile([128, 512], F32, tag="av")
                        ptA_p = ptB_p = None
                        for tt in range(ST + 1):
                            if tt < ST:
                                ks = KT[:, tt * 128 : (tt + 1) * 128]
                                pssA = scpool.tile([128, 512], F32, tag="pss")
                                nc.tensor.matmul(pssA, ks, QT[:, hA, qs],
                                                 start=True, stop=True)
                                ptA = ppool.tile([128, 512], BF16, tag="pt")
                                nc.scalar.activation(ptA, pssA, Exp)
                                pssB = scpool.tile([128, 512], F32, tag="pss")
                                nc.tensor.matmul(pssB, ks, QT[:, hB, qs],
                                                 start=True, stop=True)
                                ptB = ppool.tile([128, 512], BF16, tag="pt")
                                nc.scalar.activation(ptB, pssB, Exp)
                            if tt > 0:
                                t = tt - 1
                                st_, sp_ = (t == 0), (t == ST - 1)
                                nc.tensor.matmul(sums_A, ones, ptA_p,
                                                 start=st_, stop=sp_)
                                nc.tensor.matmul(av_A, V[:, t], ptA_p,
                                                 start=st_, stop=sp_)
                                nc.tensor.matmul(sums_B, ones, ptB_p,
                                                 start=st_, stop=sp_)
                                nc.tensor.matmul(av_B, V[:, t], ptB_p,
                                                 start=st_, stop=sp_)
                            if tt < ST:
                                ptA_p, ptB_p = ptA, ptB
                        # fast ACT drains free the PSUM banks in bank-need
                        # order (sums_A, av_A, sums_B, av_B)
                        ssA = spool.tile([128, 512], F32, tag="ssb")
                        nc.scalar.activation(ssA, sums_A, Copy)
                        avsA = avspool.tile([128, 512], BF16, tag="avsb")
                        nc.scalar.activation(avsA, av_A, Copy)
                        ssB = spool.tile([128, 512], F32, tag="ssb")
                        nc.scalar.activation(ssB, sums_B, Copy)
                        avsB = avspool.tile([128, 512], BF16, tag="avsb")
                        nc.scalar.activation(avsB, av_B, Copy)
                        # normalization entirely off the PE critical path
                        rcA = rpool.tile([128, 512], F32, tag="rc")
                        nc.vector.reciprocal(rcA, ssA)
                        nc.vector.tensor_tensor(outT[:, hA, qs], avsA, rcA,
                                                Mult)
                        rcB = rpool.tile([128, 512], F32, tag="rc")
                        nc.vector.reciprocal(rcB, ssB)
                        nc.vector.tensor_tensor(outT[:, hB, qs], avsB, rcB,
                                                Mult)

            # ---- phase 3: output projection (transposed) ----
            with tc.tile_pool(name="ostage", bufs=4) as ostage, \
                 tc.tile_pool(name="ps_o", bufs=4, space="PSUM") as ps_o:
                for et in range(ET):
                    for sc in range(SC):
                        po = ps_o.tile([128, 512], F32, tag="po")
                        for h in range(R):
                            nc.tensor.matmul(
                                po,
                                wo_sb[:, h, et * 128 : (et + 1) * 128],
                                outT[:, h, sc * 512 : (sc + 1) * 512],
                                start=(h == 0), stop=(h == R - 1),
                            )
                        st = ostage.tile([128, 512], F32, tag="ost")
                        nc.vector.tensor_copy(st, po)
                        nc.sync.dma_start(
                            otd[et * 128 : (et + 1) * 128,
                                sc * 512 : (sc + 1) * 512],
                            st,
                        )

    _split_multi_waits(nc)
    return nc


def _prepare(x, Wq, bq, Wk, bk, Wv, bv, Wo, bo):
    """Host-side sharding: build per-core input maps (bf16)."""
    import ml_dtypes

    bf16 = ml_dtypes.bfloat16
    x = np.asarray(x, dtype=np.float32)
    Wq = np.asarray(Wq, dtype=np.float32)
    bq = np.asarray(bq, dtype=np.float32)
    Wk = np.asarray(Wk, dtype=np.float32)
    bk = np.asarray(bk, dtype=np.float32)
    Wv = np.asarray(Wv, dtype=np.float32)
    bv = np.asarray(bv, dtype=np.float32)
    Wo = np.asarray(Wo, dtype=np.float32)

    isd = np.float32(1.0 / np.sqrt(D))
    xTs = [np.ascontiguousarray(x[b].T).astype(bf16) for b in range(B)]
    wqs = [
        np.ascontiguousarray(Wq[:, g * R * D : (g + 1) * R * D] * isd).astype(bf16)
        for g in range(G)
    ]
    wks = [np.ascontiguousarray(Wk[:, g * D : (g + 1) * D]).astype(bf16)
           for g in range(G)]
    wvs = [np.ascontiguousarray(Wv[:, g * D : (g + 1) * D]).astype(bf16)
           for g in range(G)]
    wos = [np.ascontiguousarray(Wo[g * R * D : (g + 1) * R * D, :]).astype(bf16)
           for g in range(G)]
    in_maps = []
    for core in range(8):
        b, g = divmod(core, G)
        in_maps.append({
            "xT": xTs[b],
            "wq": wqs[g],
            "wk": wks[g],
            "wv": wvs[g],
            "wo": wos[g],
            "bqv": bq[g * R * D : (g + 1) * R * D] * isd,
            "bkv": bk[g * D : (g + 1) * D],
            "bvv": bv[g * D : (g + 1) * D],
        })
    return in_maps


def _gather(results, bo):
    bo = np.asarray(bo, dtype=np.float32)
    out = np.empty((B, S, E), dtype=np.float32)
    for b in range(B):
        acc = results[b * G]["ot"].copy()
        for g in range(1, G):
            acc += results[b * G + g]["ot"]
        out[b] = acc.T + bo
    return out


def kernel(x, Wq, bq, Wk, bk, Wv, bv, Wo, bo):
    from concourse.bass_utils import run_bass_kernel_spmd

    if "nc" not in _cache:
        _cache["nc"] = _build_program()
    nc = _cache["nc"]
    in_maps = _prepare(x, Wq, bq, Wk, bk, Wv, bv, Wo, bo)
    res = run_bass_kernel_spmd(nc, in_maps, core_ids=list(range(8)))
    return _gather(res.results, bo)


# revision 5
# speedup vs baseline: 1.3972x; 1.0383x over previous
"""GQA attention kernel for 8 Trainium2 NeuronCores.

Sharding: core = (batch b, kv_group g), b in {0,1}, g in {0..3}.
Each core computes the 4 heads of one KV group for one batch and the
partial output projection for those heads; the host sums the 4 group
partials per batch.  Zero duplicated compute across cores.

All matmul operands are bf16 (fp32 PSUM accumulation); verified to give
~6e-3 max rel err vs the fp32 reference (tolerance 2e-2).

Performance structure (the tensor engine is the bottleneck, so the
whole kernel is organized to keep its queue dense and stall-free):
  - phase 1 (QKV projections): e-tile innermost with 6 concurrent PSUM
    accumulation groups, so compute starts after the first x/weight
    tile lands; x-tiles stream on the SP DMA queue while weights and
    the V DMA-xbar transposes ride the Activation-engine DMA queue, so
    neither queue falls behind the matmul rate.
  - phase 2 (attention): two head-streams share one [128,1024] scores
    PSUM tile (one exp per kv-tile instead of two halves the ACT
    engine's per-op overhead; ACT is the closest engine to PE
    saturation here).  The sums/AV matmuls consume exp(scores) from
    TWO kv-tiles back, so the ACT latency is hidden behind ~8 matmuls
    and the PE never waits.
  - softmax: denominators via ones-stationary matmul (partition
    reduction on PE); all PSUM drains are DVE copies ordered in
    bank-reuse order, and the slow reciprocal + normalize run
    off-critical-path on DVE against SBUF copies.
  - phase 3 (output projection): transposed accumulation; bf16 partial
    outputs, DMA'd on both queues, fully hidden behind the matmuls.
"""

import numpy as np

# problem shape (hardcoded per contract)
B, S, E = 2, 2048, 2048
H, G, D = 16, 4, 128
R = H // G          # heads per kv group = 4
ST = S // 128       # 16 t-tiles
ET = E // 128       # 16 e-tiles
SC = S // 512       # 4 s-chunks
QC = S // 512       # 4 q-chunks
NO = R + 2          # projection outputs per e-tile: 4x Q slices, K, V

_cache = {}


def _split_multi_waits(nc, maxw=1):
    """Walrus in this container accepts only one sync-wait per
    instruction; move extra waits onto preceding same-engine NoOps."""
    from concourse import mybir

    n_split = 0
    for fn in nc.m.functions:
        for bb in fn.blocks:
            out = []
            changed = False
            for inst in bb.instructions:
                si = inst.sync_info
                waits = list(si.on_wait or []) if si is not None else []
                if len(waits) > maxw:
                    changed = True
                    n_split += 1
                    head, tail = waits[:-maxw], waits[-maxw:]
                    for j in range(0, len(head), maxw):
                        nop = mybir.InstNoOp(
                            name=f"{inst.name}-wsplit{j}", ins=[], outs=[]
                        )
                        nop.engine = inst.engine
                        nop.sync_info = mybir.SyncInfo(
                            on_wait=head[j : j + maxw], on_update=[]
                        )
                        out.append(nop)
                    si.on_wait = tail
                out.append(inst)
            if changed:
                bb.instructions = out
    return n_split


def _build_program():
    import contextlib

    import concourse.bass as bass
    import concourse.tile as tile
    from concourse import mybir

    BF16 = mybir.dt.bfloat16
    F32 = mybir.dt.float32
    Exp = mybir.ActivationFunctionType.Exp
    Mult = mybir.AluOpType.mult

    nc = bass.Bass(target_bir_lowering=False)

    xT = nc.dram_tensor("xT", [E, S], BF16, kind="ExternalInput")
    wq = nc.dram_tensor("wq", [E, R * D], BF16, kind="ExternalInput")
    wk = nc.dram_tensor("wk", [E, D], BF16, kind="ExternalInput")
    wv = nc.dram_tensor("wv", [E, D], BF16, kind="ExternalInput")
    wo = nc.dram_tensor("wo", [R * D, E], BF16, kind="ExternalInput")
    bqv = nc.dram_tensor("bqv", [R * D], F32, kind="ExternalInput")
    bkv = nc.dram_tensor("bkv", [D], F32, kind="ExternalInput")
    bvv = nc.dram_tensor("bvv", [D], F32, kind="ExternalInput")
    otd = nc.dram_tensor("ot", [E, S], BF16, kind="ExternalOutput")

    with tile.TileContext(nc) as tc:
        with contextlib.ExitStack() as ctx:
            consts = ctx.enter_context(tc.tile_pool(name="consts", bufs=1))
            big = ctx.enter_context(tc.tile_pool(name="big", bufs=1))

            bq_sb = consts.tile([128, R], F32)
            nc.sync.dma_start(bq_sb, bqv.rearrange("(o p) -> p o", p=128))
            bk_sb = consts.tile([128, 1], F32)
            nc.sync.dma_start(bk_sb, bkv.rearrange("(o p) -> p o", p=128))
            bv_sb = consts.tile([128, 1], F32)
            nc.sync.dma_start(bv_sb, bvv.rearrange("(o p) -> p o", p=128))

            ones_f = consts.tile([128, 128], F32)
            nc.gpsimd.memset(ones_f, 1.0)
            ones = consts.tile([128, 128], BF16)
            nc.vector.tensor_copy(ones, ones_f)

            QT = big.tile([128, R, S], BF16)    # QT[d, h, q]
            KT = big.tile([128, S], BF16)       # KT[d, t]
            VT = big.tile([128, S], BF16)       # VT[d, t]
            V = big.tile([128, ST, D], BF16)    # V[t%128, tt, d]
            outT = big.tile([128, R, S], BF16)  # normalized attn out
            wo_sb = big.tile([128, R, E], BF16)

            # ---- phase 1: QKV^T projections ----
            with tc.tile_pool(name="wts", bufs=1) as wpool, \
                 tc.tile_pool(name="xts", bufs=2) as xtpool, \
                 tc.tile_pool(name="ps1", bufs=8, space="PSUM") as ps1:
                wq_sb = wpool.tile([128, ET, R * D], BF16)
                wk_sb = wpool.tile([128, ET, D], BF16)
                wv_sb = wpool.tile([128, ET, D], BF16)
                # weights ride the ACT-engine DMA queue; x-tiles ride the
                # SP queue, so both stay ahead of the matmul rate
                xt0 = xtpool.tile([128, ET, 512], BF16, tag="xt")
                for e in range(ET):
                    re = slice(e * 128, (e + 1) * 128)
                    nc.scalar.dma_start(wq_sb[:, e], wq[re, :])
                    nc.scalar.dma_start(wk_sb[:, e], wk[re, :])
                    nc.scalar.dma_start(wv_sb[:, e], wv[re, :])
                    nc.sync.dma_start(xt0[:, e], xT[re, 0:512])

                for sc in range(SC):
                    if sc == 0:
                        xtile = xt0
                    else:
                        xtile = xtpool.tile([128, ET, 512], BF16, tag="xt")
                        for e in range(ET):
                            nc.sync.dma_start(
                                xtile[:, e],
                                xT[e * 128 : (e + 1) * 128,
                                   sc * 512 : (sc + 1) * 512],
                            )
                    cs = slice(sc * 512, (sc + 1) * 512)
                    pss = [ps1.tile([128, 512], F32, tag="p1",
                                    name=f"p1_{sc}_{i}")
                           for i in range(NO)]
                    for e in range(ET):
                        for ot in range(NO):
                            if ot < R:
                                lhsT = wq_sb[:, e, ot * 128 : (ot + 1) * 128]
                            elif ot == R:
                                lhsT = wk_sb[:, e]
                            else:
                                lhsT = wv_sb[:, e]
                            nc.tensor.matmul(
                                pss[ot], lhsT, xtile[:, e],
                                start=(e == 0), stop=(e == ET - 1),
                            )
                    for ot in range(NO):
                        if ot < R:
                            nc.scalar.add(QT[:, ot, cs], pss[ot],
                                          bq_sb[:, ot : ot + 1])
                        elif ot == R:
                            nc.scalar.add(KT[:, cs], pss[ot], bk_sb[:, 0:1])
                        else:
                            nc.scalar.add(VT[:, cs], pss[ot], bv_sb[:, 0:1])
                    # V transpose for this chunk's 4 t-tiles via the DMA xbar
                    for tt in range(sc * 4, sc * 4 + 4):
                        nc.scalar.dma_start_transpose(
                            V[:, tt], VT[:, tt * 128 : (tt + 1) * 128]
                        )

                # wo is needed only in phase 3; SP queue is idle by then
                nc.sync.dma_start(wo_sb, wo.rearrange("(o p) m -> p o m", p=128))

            # ---- phase 2: attention, two head-streams, 2-deep pipeline ----
            with tc.tile_pool(name="probs", bufs=4) as ppool, \
                 tc.tile_pool(name="ssb", bufs=4) as spool, \
                 tc.tile_pool(name="avsb", bufs=4) as avspool, \
                 tc.tile_pool(name="rcs", bufs=4) as rpool, \
                 tc.tile_pool(name="ps_sc", bufs=2, space="PSUM") as scpool, \
                 tc.tile_pool(name="ps_sum", bufs=2, space="PSUM") as smpool, \
                 tc.tile_pool(name="ps_av", bufs=2, space="PSUM") as avpool:
                for qc in range(QC):
                    qs = slice(qc * 512, (qc + 1) * 512)
                    for hp in range(R // 2):
                        hA, hB = 2 * hp, 2 * hp + 1
                        sums_A = smpool.tile([128, 512], F32, tag="sums")
                        sums_B = smpool.tile([128, 512], F32, tag="sums")
                        av_A = avpool.tile([128, 512], F32, tag="av")
                        av_B = avpool.tile([128, 512], F32, tag="av")
                        pts = {}
                        for tt in range(ST + 2):
                            if tt < ST:
                                ks = KT[:, tt * 128 : (tt + 1) * 128]
                                # both streams' scores into one 2-bank tile
                                psc = scpool.tile([128, 2, 512], F32,
                                                  tag="pss")
                                nc.tensor.matmul(psc[:, 0], ks, QT[:, hA, qs],
                                                 start=True, stop=True)
                                nc.tensor.matmul(psc[:, 1], ks, QT[:, hB, qs],
                                                 start=True, stop=True)
                                pt = ppool.tile([128, 2, 512], BF16, tag="pt")
                                nc.scalar.activation(pt, psc, Exp)
                                pts[tt] = pt
                            if tt >= 2:
                                t = tt - 2
                                ptc = pts.pop(t)
                                st_, sp_ = (t == 0), (t == ST - 1)
                                nc.tensor.matmul(sums_A, ones, ptc[:, 0],
                                                 start=st_, stop=sp_)
                                nc.tensor.matmul(av_A, V[:, t], ptc[:, 0],
                                                 start=st_, stop=sp_)
                                nc.tensor.matmul(sums_B, ones, ptc[:, 1],
                                                 start=st_, stop=sp_)
                                nc.tensor.matmul(av_B, V[:, t], ptc[:, 1],
                                                 start=st_, stop=sp_)
                        # DVE drains in bank-reuse order free PSUM fast;
                        # reciprocal + normalize run later, off critical path
                        ssA = spool.tile([128, 512], F32, tag="ssb")
                        nc.vector.tensor_copy(ssA, sums_A)
                        avsA = avspool.tile([128, 512], BF16, tag="avsb")
                        nc.vector.tensor_copy(avsA, av_A)
                        ssB = spool.tile([128, 512], F32, tag="ssb")
                        nc.vector.tensor_copy(ssB, sums_B)
                        avsB = avspool.tile([128, 512], BF16, tag="avsb")
                        nc.vector.tensor_copy(avsB, av_B)
                        rcA = rpool.tile([128, 512], F32, tag="rc")
                        nc.vector.reciprocal(rcA, ssA)
                        nc.vector.tensor_tensor(outT[:, hA, qs], avsA, rcA,
                                                Mult)
                        rcB = rpool.tile([128, 512], F32, tag="rc")
                        nc.vector.reciprocal(rcB, ssB)
                        nc.vector.tensor_tensor(outT[:, hB, qs], avsB, rcB,
                                                Mult)

            # ---- phase 3: output projection (transposed) ----
            with tc.tile_pool(name="ostage", bufs=4) as ostage, \
                 tc.tile_pool(name="ps_o", bufs=4, space="PSUM") as ps_o:
                for et in range(ET):
                    for sc in range(SC):
                        po = ps_o.tile([128, 512], F32, tag="po")
                        for h in range(R):
                            nc.tensor.matmul(
                                po,
                                wo_sb[:, h, et * 128 : (et + 1) * 128],
                                outT[:, h, sc * 512 : (sc + 1) * 512],
                                start=(h == 0), stop=(h == R - 1),
                            )
                        st = ostage.tile([128, 512], BF16, tag="ost")
                        nc.vector.tensor_copy(st, po)
                        eng = nc.sync if (et * SC + sc) % 2 == 0 else nc.scalar
                        eng.dma_start(
                            otd[et * 128 : (et + 1) * 128,
                                sc * 512 : (sc + 1) * 512],
                            st,
                        )

    _split_multi_waits(nc)
    return nc


def _prepare(x, Wq, bq, Wk, bk, Wv, bv, Wo, bo):
    """Host-side sharding: build per-core input maps (bf16)."""
    import ml_dtypes

    bf16 = ml_dtypes.bfloat16
    x = np.asarray(x, dtype=np.float32)
    Wq = np.asarray(Wq, dtype=np.float32)
    bq = np.asarray(bq, dtype=np.float32)
    Wk = np.asarray(Wk, dtype=np.float32)
    bk = np.asarray(bk, dtype=np.float32)
    Wv = np.asarray(Wv, dtype=np.float32)
    bv = np.asarray(bv, dtype=np.float32)
    Wo = np.asarray(Wo, dtype=np.float32)

    isd = np.float32(1.0 / np.sqrt(D))
    xTs = [np.ascontiguousarray(x[b].T).astype(bf16) for b in range(B)]
    wqs = [
        np.ascontiguousarray(Wq[:, g * R * D : (g + 1) * R * D] * isd).astype(bf16)
        for g in range(G)
    ]
    wks = [np.ascontiguousarray(Wk[:, g * D : (g + 1) * D]).astype(bf16)
           for g in range(G)]
    wvs = [np.ascontiguousarray(Wv[:, g * D : (g + 1) * D]).astype(bf16)
           for g in range(G)]
    wos = [np.ascontiguousarray(Wo[g * R * D : (g + 1) * R * D, :]).astype(bf16)
           for g in range(G)]
    in_maps = []
    for core in range(8):
        b, g = divmod(core, G)
        in_maps.append({
            "xT": xTs[b],
            "wq": wqs[g],
            "wk": wks[g],
            "wv": wvs[g],
            "wo": wos[g],
            "bqv": bq[g * R * D : (g + 1) * R * D] * isd,
            "bkv": bk[g * D : (g + 1) * D],
            "bvv": bv[g * D : (g + 1) * D],
        })
    return in_maps


def _gather(results, bo):
    bo = np.asarray(bo, dtype=np.float32)
    out = np.empty((B, S, E), dtype=np.float32)
    for b in range(B):
        acc = results[b * G]["ot"].astype(np.float32)
        for g in range(1, G):
            acc += results[b * G + g]["ot"].astype(np.float32)
        out[b] = acc.T + bo
    return out


def kernel(x, Wq, bq, Wk, bk, Wv, bv, Wo, bo):
    from concourse.bass_utils import run_bass_kernel_spmd

    if "nc" not in _cache:
        _cache["nc"] = _build_program()
    nc = _cache["nc"]
    in_maps = _prepare(x, Wq, bq, Wk, bk, Wv, bv, Wo, bo)
    res = run_bass_kernel_spmd(nc, in_maps, core_ids=list(range(8)))
    return _gather(res.results, bo)
